# revision 9
# baseline (speedup 1.0000x reference)
"""Multi-head attention (batch=2, seq=2048, dim=256, nhead=8, head_dim=256)
distributed across 8 trn2 NeuronCores.

Sharding: the 16 (batch, head) pairs are distributed 2-per-core (cores 0-3
handle batch 0 heads 0-7, cores 4-7 batch 1). Each core computes its two
heads' q/k projections + attention; the output projection is FUSED into the
value projection on the host (Wvo = 16*Wo_h @ Wv_h), so the device AV matmul
directly produces the Wo-projected partial numerator in [d, sq] orientation.
Per-head numerators and softmax denominators are DMA'd out raw; the host
does the divisions, head/core sums, transpose, and bias add.

On-device per core (PSUM accumulation is always fp32):
  qT/kT [d=256, s=2048] computed bf16->fp8e4m3; QK^T runs fp8 DoubleRow
  (contraction 256 in one matmul). expT via ScalarE Exp(scale=1/16) psum->
  fp8 E (scores |s|<~16, so no max-subtraction). AV runs fp8 DoubleRow over
  sk-tile pairs against vo2 (fp8, 16x-scaled to sit in e4m3 normal range).
  Denominators: bf16 add-tree over expT sk-tiles (DVE), then one
  ones(=16)-stationary matmul per (head, chunk) -> [1, 512] psum row ->
  direct 2KB DMA. Numerator psum tiles DMA straight to DRAM (no eviction).
  The PE stream is software-pipelined: QK of chunk c+1 interleaves AV of
  chunk c (and, across heads, head j+1's QK chunk 0 interleaves head j's
  last AV); vo/proj matmuls fill the otherwise Exp-paced QK slots. Proj
  psum evictions alternate DVE/GpSimd so casts never pace the PE. A short
  burst of warmup matmuls during the input DMA window pre-ramps the PE
  clock. DMA issue is round-robined over the sync/scalar/gpsimd sequencers.
"""

import sys

if "/opt/trn_rl_repo" not in sys.path:
    sys.path.insert(0, "/opt/trn_rl_repo")

import numpy as np
import ml_dtypes

P = 128
S = 2048
D = 256
CHUNK = 512
CH = S // CHUNK  # 4 sq chunks
NKT = S // P     # 16 sk tiles
NHEAD = 8
NCORES = 8
NWARM = 16

_BUILT = None


def _build():
    import concourse.bacc as bacc
    import concourse.mybir as mybir
    import concourse.tile as tile
    from contextlib import ExitStack

    BF = mybir.dt.bfloat16
    FP8 = mybir.dt.float8e4
    F32 = mybir.dt.float32
    EXP = mybir.ActivationFunctionType.Exp
    DR = mybir.MatmulPerfMode.DoubleRow

    nc = bacc.Bacc(None, target_bir_lowering=False, debug=False)
    with tile.TileContext(nc) as tc:
        with ExitStack() as ctx:
            dram = ctx.enter_context(tc.tile_pool(name="dram", bufs=1, space="DRAM"))
            xt_d = dram.tile([2, P, S], BF, kind="ExternalInput", name="xt")
            wq_d = dram.tile([2, 2, P, D], BF, kind="ExternalInput", name="wq")
            wk_d = dram.tile([2, 2, P, D], BF, kind="ExternalInput", name="wk")
            wvo_d = dram.tile([2, P, 2 * D], BF, kind="ExternalInput", name="wvo")
            num_d = dram.tile([2, 2, P, S], BF, kind="ExternalOutput", name="num")
            den_d = dram.tile([2, CH, CHUNK], F32, kind="ExternalOutput", name="den")

            const = ctx.enter_context(tc.tile_pool(name="const", bufs=1))
            ones_bf = const.tile([P, 1], BF, name="ones_bf")
            nc.vector.memset(ones_bf[:], 16.0)
            warm_a = const.tile([P, P], BF, name="warm_a")
            nc.vector.memset(warm_a[:], 0.0)

            xpool = ctx.enter_context(tc.tile_pool(name="xtp", bufs=1))
            wpool = ctx.enter_context(tc.tile_pool(name="wp", bufs=1))
            xt_sb = [xpool.tile([P, S], BF, name=f"xt{et}") for et in range(2)]
            w_sb = {}
            for nm in ("wq", "wk"):
                for j in range(2):
                    for et in range(2):
                        w_sb[(nm, j, et)] = wpool.tile([P, D], BF, name=f"{nm}{j}{et}")
            wvo_sb = [wpool.tile([P, 2 * D], BF, name=f"wvo{et}") for et in range(2)]

            # ---- input DMAs: priority order (first compute needs wk/wq j0 +
            # xt chunk 0), issue round-robined over 3 DMA-capable sequencers.
            # First-needed tensors are split fine so they land on many queues.
            dma_engines = [nc.sync, nc.scalar, nc.gpsimd]
            loads = []
            Q = CHUNK // 4
            for q4 in range(4):
                for et in range(2):
                    loads.append((xt_sb[et][:, q4 * Q:(q4 + 1) * Q],
                                  xt_d[et, :, q4 * Q:(q4 + 1) * Q]))
            for et in range(2):
                loads.append((w_sb[("wk", 0, et)][:], wk_d[0, et]))
            for et in range(2):
                loads.append((w_sb[("wq", 0, et)][:], wq_d[0, et]))
            for c in range(1, CH):
                for et in range(2):
                    loads.append((xt_sb[et][:, c * CHUNK:(c + 1) * CHUNK],
                                  xt_d[et, :, c * CHUNK:(c + 1) * CHUNK]))
            for et in range(2):
                loads.append((wvo_sb[et][:], wvo_d[et]))
            for nm, src in (("wk", wk_d), ("wq", wq_d)):
                for et in range(2):
                    loads.append((w_sb[(nm, 1, et)][:], src[1, et]))
            for i, (dst, srcap) in enumerate(loads):
                dma_engines[i % 3].dma_start(out=dst, in_=srcap)

            qkpool = ctx.enter_context(tc.tile_pool(name="qkp", bufs=2))
            vpool = ctx.enter_context(tc.tile_pool(name="vp", bufs=1))
            epool = ctx.enter_context(tc.tile_pool(name="ep", bufs=3))
            tpool = ctx.enter_context(tc.tile_pool(name="tp", bufs=2))

            psA = ctx.enter_context(tc.tile_pool(name="psA", bufs=3, space="PSUM"))
            psB = ctx.enter_context(tc.tile_pool(name="psB", bufs=2, space="PSUM"))
            psV = ctx.enter_context(tc.tile_pool(name="psV", bufs=2, space="PSUM"))
            psD = ctx.enter_context(tc.tile_pool(name="psD", bufs=1, space="PSUM"))

            # ---- PE warmup: garbage matmuls on a memset tile during the
            # input-DMA window, to pre-ramp the PE clock
            for w in range(NWARM):
                ps = psB.tile([P, CHUNK], F32, tag="psB", name="ps_warm")
                nc.tensor.matmul(ps[:, :P], lhsT=warm_a[:], rhs=warm_a[:],
                                 start=True, stop=True)

            # fused vo projection for BOTH heads at once:
            # vo2[s, h*256+o] fp8 (16x-scaled); layout [P, kt*512]
            vo2_sb = vpool.tile([P, NKT * 2 * D], FP8, name="vo2")
            vo2r = vo2_sb.rearrange("p (kt w) -> p kt w", kt=NKT)

            # filler machinery: each filler() emits ONE PE op (plus its
            # eviction on an alternating DVE/gpsimd engine)
            cast_rr = [0]

            def evict(dst, src_ap):
                # rotate psum evictions 3:1 over DVE:ScalarE (gpsimd cannot
                # read PSUM); ScalarE also carries the Exp load
                if cast_rr[0] % 4 == 3:
                    nc.scalar.copy(dst, src_ap)
                else:
                    nc.vector.tensor_copy(dst, src_ap)
                cast_rr[0] += 1

            def emit_vo_st(st):
                ps = psB.tile([P, CHUNK], F32, tag="psB", name="ps_vo")
                for et in range(2):
                    nc.tensor.matmul(
                        ps[:],
                        lhsT=xt_sb[et][:, st * P:(st + 1) * P],
                        rhs=wvo_sb[et][:],
                        start=(et == 0), stop=(et == 1),
                    )
                evict(vo2_sb[:, st * 2 * D:(st + 1) * 2 * D], ps[:])

            def emit_proj_group(j, wname, dst, c, dt):
                ps = psB.tile([P, CHUNK], F32, tag="psB", name="ps_proj")
                for et in range(2):
                    nc.tensor.matmul(
                        ps[:],
                        lhsT=w_sb[(wname, j, et)][:, dt * P:(dt + 1) * P],
                        rhs=xt_sb[et][:, c * CHUNK:(c + 1) * CHUNK],
                        start=(et == 0), stop=(et == 1),
                    )
                evict(dst[:, dt * S + c * CHUNK: dt * S + (c + 1) * CHUNK],
                      ps[:])

            def make_proj_fillers(j, qt_sb, kt_sb):
                fillers = []
                for c in range(CH):
                    for dst, wname in ((kt_sb, "wk"), (qt_sb, "wq")):
                        for dt in range(2):
                            fillers.append(
                                (emit_proj_group, (j, wname, dst, c, dt)))
                return fillers

            def alloc_qkt(j):
                qt_sb = qkpool.tile([P, 2 * S], FP8, tag="qt", name=f"qt_{j}")
                kt_sb = qkpool.tile([P, 2 * S], FP8, tag="kt", name=f"kt_{j}")
                return qt_sb, kt_sb

            def emit_tree(j, c, E):
                t1 = tpool.tile([P, 8 * CHUNK], BF, tag="t1", name="t1")
                nc.gpsimd.tensor_add(t1[:], E[:, :8 * CHUNK], E[:, 8 * CHUNK:])
                t2 = tpool.tile([P, 4 * CHUNK], BF, tag="t2", name="t2")
                nc.vector.tensor_add(t2[:], t1[:, :4 * CHUNK], t1[:, 4 * CHUNK:])
                t3 = tpool.tile([P, 2 * CHUNK], BF, tag="t3", name="t3")
                nc.vector.tensor_add(t3[:], t2[:, :2 * CHUNK], t2[:, 2 * CHUNK:])
                t4 = tpool.tile([P, CHUNK], BF, tag="t4", name="t4")
                nc.vector.tensor_add(t4[:], t3[:, :CHUNK], t3[:, CHUNK:])
                return t4

            dnpool = ctx.enter_context(tc.tile_pool(name="dnp", bufs=2))
            numpool = ctx.enter_context(tc.tile_pool(name="nump", bufs=4))

            def emit_densum(j, c, t4):
                psd = psD.tile([P, CHUNK], F32, tag="psD", name="ps_d")
                nc.tensor.matmul(psd[0:1, :], lhsT=ones_bf[:], rhs=t4[:],
                                 start=True, stop=True)
                den_sb = dnpool.tile([1, CHUNK], F32, tag="den", name="den_sb")
                nc.vector.tensor_copy(den_sb[:], psd[0:1, :])
                dma_engines[(j * CH + c) % 3].dma_start(
                    out=den_d[j, c], in_=den_sb[:])

            # one g-step of the interleaved PE stream: 2 QK matmuls (+Exp)
            # of chunk c, then one AV DR pair-group of av_spec, then fillers
            def emit_chunk(j, c, qt3, kt3, E_c, av_spec, fillers, nfill):
                if av_spec is not None:
                    av_j, E_prev, psv = av_spec
                    Er = E_prev.rearrange("p (kt s) -> p kt s", kt=NKT)
                for g in range(NKT // 2):
                    for half in range(2):
                        kt_idx = 2 * g + half
                        ps = psA.tile([P, CHUNK], F32, tag="psA", name="ps_qk")
                        nc.tensor.matmul(
                            ps[:],
                            lhsT=kt3[:, :, kt_idx * P:(kt_idx + 1) * P],
                            rhs=qt3[:, :, c * CHUNK:(c + 1) * CHUNK],
                            start=True, stop=True, perf_mode=DR,
                        )
                        nc.scalar.activation(
                            E_c[:, kt_idx * CHUNK:(kt_idx + 1) * CHUNK], ps[:],
                            EXP, scale=1.0 / 16.0,
                        )
                    if av_spec is not None:
                        for dt in range(2):
                            off = av_j * D + dt * P
                            nc.tensor.matmul(
                                psv[dt][:],
                                lhsT=vo2r[:, 2 * g:2 * g + 2, off:off + P],
                                rhs=Er[:, 2 * g:2 * g + 2, :],
                                start=(g == 0), stop=(g == NKT // 2 - 1),
                                perf_mode=DR,
                            )
                    for _ in range(nfill):
                        if fillers:
                            fn, args = fillers.pop(0)
                            fn(*args)

            def emit_av_flush(av_j, av_c, E_prev, psv):
                # stop'd AV psum tiles -> SBUF (alternating engines) -> DRAM
                for dt in range(2):
                    nsb = numpool.tile([P, CHUNK], BF, tag="num", name="num_sb")
                    evict(nsb[:], psv[dt][:])
                    dma_engines[(av_c * 2 + dt) % 3].dma_start(
                        out=num_d[av_j, dt, :, av_c * CHUNK:(av_c + 1) * CHUNK],
                        in_=nsb[:])

            def emit_av_alone(av_j, av_c, E_prev, psv):
                Er = E_prev.rearrange("p (kt s) -> p kt s", kt=NKT)
                for g in range(NKT // 2):
                    for dt in range(2):
                        off = av_j * D + dt * P
                        nc.tensor.matmul(
                            psv[dt][:],
                            lhsT=vo2r[:, 2 * g:2 * g + 2, off:off + P],
                            rhs=Er[:, 2 * g:2 * g + 2, :],
                            start=(g == 0), stop=(g == NKT // 2 - 1),
                            perf_mode=DR,
                        )

            # ================= emission =================
            # proj head 0 (not fillable: nothing else to do yet)
            qt0, kt0 = alloc_qkt(0)
            for fn, args in make_proj_fillers(0, qt0, kt0):
                fn(*args)
            # first half of vo upfront; rest fills head-0 chunk-0 QK slots
            vo_fillers = [(emit_vo_st, (st,)) for st in range(NKT)]
            for fn, args in vo_fillers[:8]:
                fn(*args)
            vo_fillers = vo_fillers[8:]

            qt3_0 = qt0.rearrange("p (ko s) -> p ko s", ko=2)
            kt3_0 = kt0.rearrange("p (ko s) -> p ko s", ko=2)
            qt1, kt1 = alloc_qkt(1)
            proj1_fillers = make_proj_fillers(1, qt1, kt1)

            E_tiles = {}

            def run_head(j, qt3, kt3, carry, fillers_by_chunk):
                # carry: (av_j, av_c, E_prev, psv) AV work interleaved into
                # chunk 0, from the previous head (or None)
                for c in range(CH):
                    E_c = epool.tile([P, NKT * CHUNK], FP8, tag="E",
                                     name=f"E_{j}_{c}")
                    E_tiles[(j, c)] = E_c
                    if c == 0:
                        av_spec = (carry[0], carry[2], carry[3]) if carry else None
                    else:
                        psv = [psV.tile([P, CHUNK], F32, tag="psv",
                                        name=f"psv{dt}_{j}_{c-1}")
                               for dt in range(2)]
                        av_spec = (j, E_tiles[(j, c - 1)], psv)
                    if c >= 2:
                        t4 = emit_tree(j, c - 2, E_tiles[(j, c - 2)])
                    fillers, nfill = fillers_by_chunk[c]
                    emit_chunk(j, c, qt3, kt3, E_c, av_spec, fillers, nfill)
                    if av_spec is not None:
                        if c == 0:
                            emit_av_flush(carry[0], carry[1], carry[2], carry[3])
                        else:
                            emit_av_flush(j, c - 1, E_tiles[(j, c - 1)], psv)
                    if c >= 2:
                        emit_densum(j, c - 2, t4)
                # trees/densums for the last two chunks
                t4 = emit_tree(j, CH - 2, E_tiles[(j, CH - 2)])
                emit_densum(j, CH - 2, t4)
                t4 = emit_tree(j, CH - 1, E_tiles[(j, CH - 1)])
                emit_densum(j, CH - 1, t4)

            # head 0: vo fills chunk 0; proj-j1 spreads over chunks 1..3
            h0_fillers = {
                0: (vo_fillers, 2),
                1: (proj1_fillers, 2),
                2: (proj1_fillers, 2),
                3: (proj1_fillers, 2),
            }
            run_head(0, qt3_0, kt3_0, None, h0_fillers)

            qt3_1 = qt1.rearrange("p (ko s) -> p ko s", ko=2)
            kt3_1 = kt1.rearrange("p (ko s) -> p ko s", ko=2)
            # leftover proj-j1 groups (if any) run before head 1
            while proj1_fillers:
                fn, args = proj1_fillers.pop(0)
                fn(*args)

            # head 1: chunk 0 interleaves head 0's last AV chunk
            psv_c = [psV.tile([P, CHUNK], F32, tag="psv", name=f"psv{dt}_0_3")
                     for dt in range(2)]
            carry = (0, CH - 1, E_tiles[(0, CH - 1)], psv_c)
            h1_fillers = {c: ([], 0) for c in range(CH)}
            run_head(1, qt3_1, kt3_1, carry, h1_fillers)

            # final AV chunk of head 1 (nothing left to interleave)
            psv_f = [psV.tile([P, CHUNK], F32, tag="psv", name=f"psv{dt}_1_3")
                     for dt in range(2)]
            emit_av_alone(1, CH - 1, E_tiles[(1, CH - 1)], psv_f)
            emit_av_flush(1, CH - 1, E_tiles[(1, CH - 1)], psv_f)
    nc.compile()
    names = dict(xt=xt_d.name, wq=wq_d.name, wk=wk_d.name, wvo=wvo_d.name,
                 num=num_d.name, den=den_d.name)
    return nc, names


def _get_built():
    global _BUILT
    if _BUILT is None:
        _BUILT = _build()
    return _BUILT


def _prep_core_inputs(i, x, Wq, Wk, Wvo, names):
    bf16 = ml_dtypes.bfloat16
    b = i // 4
    heads = [(2 * i) % NHEAD, (2 * i) % NHEAD + 1]
    xt = np.ascontiguousarray(x[b].T).reshape(2, P, S).astype(bf16)

    def head_T(W, h):  # W[h*D:(h+1)*D, :].T -> [e=256, d=256] -> [2,128,256]
        return np.ascontiguousarray(W[h * D:(h + 1) * D, :].T).reshape(2, P, D)

    wq = np.stack([head_T(Wq, h) for h in heads]).astype(bf16)
    wk = np.stack([head_T(Wk, h) for h in heads]).astype(bf16)
    # wvo: both heads side by side -> [et=2, 128, 2*D]
    wvo = np.concatenate(
        [np.ascontiguousarray(Wvo[h].T).reshape(2, P, D) for h in heads],
        axis=2).astype(bf16)
    return {names["xt"]: xt, names["wq"]: wq, names["wk"]: wk,
            names["wvo"]: wvo}


def kernel(x, Wq, Wk, Wv, Wo, bo):
    from concourse.bass_utils import run_bass_kernel_spmd

    x = np.asarray(x, dtype=np.float32)
    Wq = np.asarray(Wq, dtype=np.float32)
    Wk = np.asarray(Wk, dtype=np.float32)
    Wv = np.asarray(Wv, dtype=np.float32)
    Wo = np.asarray(Wo, dtype=np.float32)
    bo = np.asarray(bo, dtype=np.float32)

    # host-fused, 16x-scaled output-projected value weights per head:
    # vo_h = x @ Wvo_h^T with Wvo_h = 16 * Wo_h @ Wv_h  -> [nhead, 256, 256]
    Wvo = np.stack([
        16.0 * (Wo[:, h * D:(h + 1) * D] @ Wv[h * D:(h + 1) * D])
        for h in range(NHEAD)
    ])

    nc, names = _get_built()
    in_maps = [_prep_core_inputs(i, x, Wq, Wk, Wvo, names) for i in range(NCORES)]
    res = run_bass_kernel_spmd(nc, in_maps, core_ids=list(range(NCORES)))

    out = np.zeros((2, S, D), dtype=np.float32)
    for b in range(2):
        acc = np.zeros((D, S), dtype=np.float32)
        for i in range(4 * b, 4 * b + 4):
            num = res.results[i][names["num"]]   # [2, 2, 128, 2048]
            den = res.results[i][names["den"]]   # [2, 4, 512]
            for j in range(2):
                acc += num[j].reshape(D, S) / den[j].reshape(S)[None, :]
        out[b] = acc.T + bo[None, :]
    return out


# revision 10
# speedup vs baseline: 1.2328x; 1.2328x over previous
"""Multi-head attention (batch=2, seq=2048, dim=256, nhead=8, head_dim=256)
distributed across 8 trn2 NeuronCores.

Sharding: the 16 (batch, head) pairs are distributed 2-per-core (cores 0-3
handle batch 0 heads 0-7, cores 4-7 batch 1). Each core computes its two
heads' q/k projections + attention; the output projection is FUSED into the
value projection on the host (Wvo = 16*Wo_h @ Wv_h), so the device AV matmul
directly produces the Wo-projected partial numerator in [d, sq] orientation.
Per-head numerators and softmax denominators are DMA'd out raw; the host
does the divisions, head/core sums, transpose, and bias add.

On-device per core (PSUM accumulation is always fp32):
  qT/kT [d=256, s=2048] computed bf16->fp8e4m3; QK^T runs fp8 DoubleRow
  (contraction 256 in one matmul). expT via ScalarE Exp(scale=1/16) psum->
  fp8 E (scores |s|<~16, so no max-subtraction). AV runs fp8 DoubleRow over
  sk-tile pairs against vo2 (fp8, 16x-scaled to sit in e4m3 normal range).
  Denominators: bf16 add-tree over expT sk-tiles (DVE), then one
  ones(=16)-stationary matmul per (head, chunk) -> [1, 512] psum row ->
  direct 2KB DMA. Numerator psum tiles DMA straight to DRAM (no eviction).
  The PE stream is software-pipelined: QK of chunk c+1 interleaves AV of
  chunk c (and, across heads, head j+1's QK chunk 0 interleaves head j's
  last AV); vo/proj matmuls fill the otherwise Exp-paced QK slots. Proj
  psum evictions alternate DVE/GpSimd so casts never pace the PE. A short
  burst of warmup matmuls during the input DMA window pre-ramps the PE
  clock. DMA issue is round-robined over the sync/scalar/gpsimd sequencers.
"""

import sys

if "/opt/trn_rl_repo" not in sys.path:
    sys.path.insert(0, "/opt/trn_rl_repo")

import numpy as np
import ml_dtypes

P = 128
S = 2048
D = 256
CHUNK = 512
CH = S // CHUNK  # 4 sq chunks
NKT = S // P     # 16 sk tiles
NHEAD = 8
NCORES = 8
NWARM = 16

_BUILT = None


def _build():
    import concourse.bacc as bacc
    import concourse.mybir as mybir
    import concourse.tile as tile
    from contextlib import ExitStack

    BF = mybir.dt.bfloat16
    FP8 = mybir.dt.float8e4
    F32 = mybir.dt.float32
    EXP = mybir.ActivationFunctionType.Exp
    DR = mybir.MatmulPerfMode.DoubleRow

    nc = bacc.Bacc(None, target_bir_lowering=False, debug=False)
    with tile.TileContext(nc) as tc:
        with ExitStack() as ctx:
            dram = ctx.enter_context(tc.tile_pool(name="dram", bufs=1, space="DRAM"))
            xt_d = dram.tile([2, P, S], BF, kind="ExternalInput", name="xt")
            wq_d = dram.tile([2, 2, P, D], BF, kind="ExternalInput", name="wq")
            wk_d = dram.tile([2, 2, P, D], BF, kind="ExternalInput", name="wk")
            wvo_d = dram.tile([2, P, 2 * D], BF, kind="ExternalInput", name="wvo")
            num_d = dram.tile([2, 2, P, S], BF, kind="ExternalOutput", name="num")
            den_d = dram.tile([2, CH, CHUNK], F32, kind="ExternalOutput", name="den")

            const = ctx.enter_context(tc.tile_pool(name="const", bufs=1))
            ones_bf = const.tile([P, 1], BF, name="ones_bf")
            nc.vector.memset(ones_bf[:], 16.0)
            warm_a = const.tile([P, P], BF, name="warm_a")
            nc.vector.memset(warm_a[:], 0.0)

            xpool = ctx.enter_context(tc.tile_pool(name="xtp", bufs=1))
            wpool = ctx.enter_context(tc.tile_pool(name="wp", bufs=1))
            xt_sb = [xpool.tile([P, S], BF, name=f"xt{et}") for et in range(2)]
            w_sb = {}
            for nm in ("wq", "wk"):
                for j in range(2):
                    for et in range(2):
                        w_sb[(nm, j, et)] = wpool.tile([P, D], BF, name=f"{nm}{j}{et}")
            wvo_sb = [wpool.tile([P, 2 * D], BF, name=f"wvo{et}") for et in range(2)]

            # ---- input DMAs: priority order (first compute needs wk/wq j0 +
            # xt chunk 0), issue round-robined over 3 DMA-capable sequencers.
            # First-needed tensors are split fine so they land on many queues.
            dma_engines = [nc.sync, nc.scalar, nc.gpsimd]
            out_dma_engines = [nc.sync, nc.gpsimd]
            loads = []
            Q = CHUNK // 4
            for q4 in range(4):
                for et in range(2):
                    loads.append((xt_sb[et][:, q4 * Q:(q4 + 1) * Q],
                                  xt_d[et, :, q4 * Q:(q4 + 1) * Q]))
            for et in range(2):
                loads.append((w_sb[("wk", 0, et)][:], wk_d[0, et]))
            for et in range(2):
                loads.append((w_sb[("wq", 0, et)][:], wq_d[0, et]))
            for c in range(1, CH):
                for et in range(2):
                    loads.append((xt_sb[et][:, c * CHUNK:(c + 1) * CHUNK],
                                  xt_d[et, :, c * CHUNK:(c + 1) * CHUNK]))
            for et in range(2):
                loads.append((wvo_sb[et][:], wvo_d[et]))
            for nm, src in (("wk", wk_d), ("wq", wq_d)):
                for et in range(2):
                    loads.append((w_sb[(nm, 1, et)][:], src[1, et]))
            for i, (dst, srcap) in enumerate(loads):
                dma_engines[i % 3].dma_start(out=dst, in_=srcap)

            qkpool = ctx.enter_context(tc.tile_pool(name="qkp", bufs=2))
            vpool = ctx.enter_context(tc.tile_pool(name="vp", bufs=1))
            epool = ctx.enter_context(tc.tile_pool(name="ep", bufs=3))
            tpool = ctx.enter_context(tc.tile_pool(name="tp", bufs=2))

            psA = ctx.enter_context(tc.tile_pool(name="psA", bufs=3, space="PSUM"))
            psB = ctx.enter_context(tc.tile_pool(name="psB", bufs=2, space="PSUM"))
            psV = ctx.enter_context(tc.tile_pool(name="psV", bufs=2, space="PSUM"))
            psD = ctx.enter_context(tc.tile_pool(name="psD", bufs=1, space="PSUM"))

            # ---- PE warmup: garbage matmuls on a memset tile during the
            # input-DMA window, to pre-ramp the PE clock
            for w in range(NWARM):
                ps = psB.tile([P, CHUNK], F32, tag="psB", name="ps_warm")
                nc.tensor.matmul(ps[:, :P], lhsT=warm_a[:], rhs=warm_a[:],
                                 start=True, stop=True)

            # fused vo projection for BOTH heads at once:
            # vo2[s, h*256+o] fp8 (16x-scaled); layout [P, kt*512]
            vo2_sb = vpool.tile([P, NKT * 2 * D], FP8, name="vo2")
            vo2r = vo2_sb.rearrange("p (kt w) -> p kt w", kt=NKT)

            # filler machinery: each filler() emits ONE PE op (plus its
            # eviction on an alternating DVE/gpsimd engine)
            cast_rr = [0]

            def evict(dst, src_ap):
                # rotate psum evictions 3:1 over DVE:ScalarE (gpsimd cannot
                # read PSUM); ScalarE also carries the Exp load
                if cast_rr[0] % 4 == 3:
                    nc.scalar.copy(dst, src_ap)
                else:
                    nc.vector.tensor_copy(dst, src_ap)
                cast_rr[0] += 1

            def emit_vo_st(st):
                ps = psB.tile([P, CHUNK], F32, tag="psB", name="ps_vo")
                for et in range(2):
                    nc.tensor.matmul(
                        ps[:],
                        lhsT=xt_sb[et][:, st * P:(st + 1) * P],
                        rhs=wvo_sb[et][:],
                        start=(et == 0), stop=(et == 1),
                    )
                evict(vo2_sb[:, st * 2 * D:(st + 1) * 2 * D], ps[:])

            def emit_proj_group(j, wname, dst, c, dt):
                ps = psB.tile([P, CHUNK], F32, tag="psB", name="ps_proj")
                for et in range(2):
                    nc.tensor.matmul(
                        ps[:],
                        lhsT=w_sb[(wname, j, et)][:, dt * P:(dt + 1) * P],
                        rhs=xt_sb[et][:, c * CHUNK:(c + 1) * CHUNK],
                        start=(et == 0), stop=(et == 1),
                    )
                evict(dst[:, dt * S + c * CHUNK: dt * S + (c + 1) * CHUNK],
                      ps[:])

            def make_proj_fillers(j, qt_sb, kt_sb):
                fillers = []
                for c in range(CH):
                    for dst, wname in ((kt_sb, "wk"), (qt_sb, "wq")):
                        for dt in range(2):
                            fillers.append(
                                (emit_proj_group, (j, wname, dst, c, dt)))
                return fillers

            def alloc_qkt(j):
                qt_sb = qkpool.tile([P, 2 * S], FP8, tag="qt", name=f"qt_{j}")
                kt_sb = qkpool.tile([P, 2 * S], FP8, tag="kt", name=f"kt_{j}")
                return qt_sb, kt_sb

            def emit_tree(j, c, E):
                t1 = tpool.tile([P, 8 * CHUNK], BF, tag="t1", name="t1")
                nc.vector.tensor_add(t1[:], E[:, :8 * CHUNK], E[:, 8 * CHUNK:])
                t2 = tpool.tile([P, 4 * CHUNK], BF, tag="t2", name="t2")
                nc.vector.tensor_add(t2[:], t1[:, :4 * CHUNK], t1[:, 4 * CHUNK:])
                t3 = tpool.tile([P, 2 * CHUNK], BF, tag="t3", name="t3")
                nc.vector.tensor_add(t3[:], t2[:, :2 * CHUNK], t2[:, 2 * CHUNK:])
                t4 = tpool.tile([P, CHUNK], BF, tag="t4", name="t4")
                nc.vector.tensor_add(t4[:], t3[:, :CHUNK], t3[:, CHUNK:])
                return t4

            dnpool = ctx.enter_context(tc.tile_pool(name="dnp", bufs=2))
            numpool = ctx.enter_context(tc.tile_pool(name="nump", bufs=4))

            def emit_densum(j, c, t4):
                psd = psD.tile([P, CHUNK], F32, tag="psD", name="ps_d")
                nc.tensor.matmul(psd[0:1, :], lhsT=ones_bf[:], rhs=t4[:],
                                 start=True, stop=True)
                den_sb = dnpool.tile([1, CHUNK], F32, tag="den", name="den_sb")
                nc.vector.tensor_copy(den_sb[:], psd[0:1, :])
                out_dma_engines[(j * CH + c) % 2].dma_start(
                    out=den_d[j, c], in_=den_sb[:])

            # one g-step of the interleaved PE stream: 2 QK matmuls (+Exp)
            # of chunk c, then one AV DR pair-group of av_spec, then fillers
            def emit_chunk(j, c, qt3, kt3, E_c, av_spec, fillers, nfill):
                if av_spec is not None:
                    av_j, E_prev, psv = av_spec
                    Er = E_prev.rearrange("p (kt s) -> p kt s", kt=NKT)
                for g in range(NKT // 2):
                    for half in range(2):
                        kt_idx = 2 * g + half
                        ps = psA.tile([P, CHUNK], F32, tag="psA", name="ps_qk")
                        nc.tensor.matmul(
                            ps[:],
                            lhsT=kt3[:, :, kt_idx * P:(kt_idx + 1) * P],
                            rhs=qt3[:, :, c * CHUNK:(c + 1) * CHUNK],
                            start=True, stop=True, perf_mode=DR,
                        )
                        nc.scalar.activation(
                            E_c[:, kt_idx * CHUNK:(kt_idx + 1) * CHUNK], ps[:],
                            EXP, scale=1.0 / 16.0,
                        )
                    if av_spec is not None:
                        for dt in range(2):
                            off = av_j * D + dt * P
                            nc.tensor.matmul(
                                psv[dt][:],
                                lhsT=vo2r[:, 2 * g:2 * g + 2, off:off + P],
                                rhs=Er[:, 2 * g:2 * g + 2, :],
                                start=(g == 0), stop=(g == NKT // 2 - 1),
                                perf_mode=DR,
                            )
                    for _ in range(nfill):
                        if fillers:
                            fn, args = fillers.pop(0)
                            fn(*args)

            def emit_av_flush(av_j, av_c, E_prev, psv):
                # stop'd AV psum tiles -> SBUF (alternating engines) -> DRAM
                for dt in range(2):
                    nsb = numpool.tile([P, CHUNK], BF, tag="num", name="num_sb")
                    evict(nsb[:], psv[dt][:])
                    out_dma_engines[(av_c * 2 + dt) % 2].dma_start(
                        out=num_d[av_j, dt, :, av_c * CHUNK:(av_c + 1) * CHUNK],
                        in_=nsb[:])

            def emit_av_alone(av_j, av_c, E_prev, psv):
                Er = E_prev.rearrange("p (kt s) -> p kt s", kt=NKT)
                for g in range(NKT // 2):
                    for dt in range(2):
                        off = av_j * D + dt * P
                        nc.tensor.matmul(
                            psv[dt][:],
                            lhsT=vo2r[:, 2 * g:2 * g + 2, off:off + P],
                            rhs=Er[:, 2 * g:2 * g + 2, :],
                            start=(g == 0), stop=(g == NKT // 2 - 1),
                            perf_mode=DR,
                        )

            # ================= emission =================
            # proj head 0 (not fillable: nothing else to do yet)
            qt0, kt0 = alloc_qkt(0)
            for fn, args in make_proj_fillers(0, qt0, kt0):
                fn(*args)
            # first half of vo upfront; rest fills head-0 chunk-0 QK slots
            vo_fillers = [(emit_vo_st, (st,)) for st in range(NKT)]
            for fn, args in vo_fillers[:8]:
                fn(*args)
            vo_fillers = vo_fillers[8:]

            qt3_0 = qt0.rearrange("p (ko s) -> p ko s", ko=2)
            kt3_0 = kt0.rearrange("p (ko s) -> p ko s", ko=2)
            qt1, kt1 = alloc_qkt(1)
            proj1_fillers = make_proj_fillers(1, qt1, kt1)

            E_tiles = {}

            def run_head(j, qt3, kt3, carry, fillers_by_chunk):
                # carry: (av_j, av_c, E_prev, psv) AV work interleaved into
                # chunk 0, from the previous head (or None)
                for c in range(CH):
                    E_c = epool.tile([P, NKT * CHUNK], FP8, tag="E",
                                     name=f"E_{j}_{c}")
                    E_tiles[(j, c)] = E_c
                    if c == 0:
                        av_spec = (carry[0], carry[2], carry[3]) if carry else None
                    else:
                        psv = [psV.tile([P, CHUNK], F32, tag="psv",
                                        name=f"psv{dt}_{j}_{c-1}")
                               for dt in range(2)]
                        av_spec = (j, E_tiles[(j, c - 1)], psv)
                    if c >= 2:
                        t4 = emit_tree(j, c - 2, E_tiles[(j, c - 2)])
                    fillers, nfill = fillers_by_chunk[c]
                    emit_chunk(j, c, qt3, kt3, E_c, av_spec, fillers, nfill)
                    if av_spec is not None:
                        if c == 0:
                            emit_av_flush(carry[0], carry[1], carry[2], carry[3])
                        else:
                            emit_av_flush(j, c - 1, E_tiles[(j, c - 1)], psv)
                    if c >= 2:
                        emit_densum(j, c - 2, t4)
                # trees/densums for the last two chunks
                t4 = emit_tree(j, CH - 2, E_tiles[(j, CH - 2)])
                emit_densum(j, CH - 2, t4)
                t4 = emit_tree(j, CH - 1, E_tiles[(j, CH - 1)])
                emit_densum(j, CH - 1, t4)

            # head 0: vo fills chunk 0; proj-j1 spreads over chunks 1..3
            h0_fillers = {
                0: (vo_fillers, 2),
                1: (proj1_fillers, 2),
                2: (proj1_fillers, 2),
                3: (proj1_fillers, 2),
            }
            run_head(0, qt3_0, kt3_0, None, h0_fillers)

            qt3_1 = qt1.rearrange("p (ko s) -> p ko s", ko=2)
            kt3_1 = kt1.rearrange("p (ko s) -> p ko s", ko=2)
            # leftover proj-j1 groups (if any) run before head 1
            while proj1_fillers:
                fn, args = proj1_fillers.pop(0)
                fn(*args)

            # head 1: chunk 0 interleaves head 0's last AV chunk
            psv_c = [psV.tile([P, CHUNK], F32, tag="psv", name=f"psv{dt}_0_3")
                     for dt in range(2)]
            carry = (0, CH - 1, E_tiles[(0, CH - 1)], psv_c)
            h1_fillers = {c: ([], 0) for c in range(CH)}
            run_head(1, qt3_1, kt3_1, carry, h1_fillers)

            # final AV chunk of head 1 (nothing left to interleave)
            psv_f = [psV.tile([P, CHUNK], F32, tag="psv", name=f"psv{dt}_1_3")
                     for dt in range(2)]
            emit_av_alone(1, CH - 1, E_tiles[(1, CH - 1)], psv_f)
            emit_av_flush(1, CH - 1, E_tiles[(1, CH - 1)], psv_f)
    nc.compile()
    names = dict(xt=xt_d.name, wq=wq_d.name, wk=wk_d.name, wvo=wvo_d.name,
                 num=num_d.name, den=den_d.name)
    return nc, names


def _get_built():
    global _BUILT
    if _BUILT is None:
        _BUILT = _build()
    return _BUILT


def _prep_core_inputs(i, x, Wq, Wk, Wvo, names):
    bf16 = ml_dtypes.bfloat16
    b = i // 4
    heads = [(2 * i) % NHEAD, (2 * i) % NHEAD + 1]
    xt = np.ascontiguousarray(x[b].T).reshape(2, P, S).astype(bf16)

    def head_T(W, h):  # W[h*D:(h+1)*D, :].T -> [e=256, d=256] -> [2,128,256]
        return np.ascontiguousarray(W[h * D:(h + 1) * D, :].T).reshape(2, P, D)

    wq = np.stack([head_T(Wq, h) for h in heads]).astype(bf16)
    wk = np.stack([head_T(Wk, h) for h in heads]).astype(bf16)
    # wvo: both heads side by side -> [et=2, 128, 2*D]
    wvo = np.concatenate(
        [np.ascontiguousarray(Wvo[h].T).reshape(2, P, D) for h in heads],
        axis=2).astype(bf16)
    return {names["xt"]: xt, names["wq"]: wq, names["wk"]: wk,
            names["wvo"]: wvo}


def kernel(x, Wq, Wk, Wv, Wo, bo):
    from concourse.bass_utils import run_bass_kernel_spmd

    x = np.asarray(x, dtype=np.float32)
    Wq = np.asarray(Wq, dtype=np.float32)
    Wk = np.asarray(Wk, dtype=np.float32)
    Wv = np.asarray(Wv, dtype=np.float32)
    Wo = np.asarray(Wo, dtype=np.float32)
    bo = np.asarray(bo, dtype=np.float32)

    # host-fused, 16x-scaled output-projected value weights per head:
    # vo_h = x @ Wvo_h^T with Wvo_h = 16 * Wo_h @ Wv_h  -> [nhead, 256, 256]
    Wvo = np.stack([
        16.0 * (Wo[:, h * D:(h + 1) * D] @ Wv[h * D:(h + 1) * D])
        for h in range(NHEAD)
    ])

    nc, names = _get_built()
    in_maps = [_prep_core_inputs(i, x, Wq, Wk, Wvo, names) for i in range(NCORES)]
    res = run_bass_kernel_spmd(nc, in_maps, core_ids=list(range(NCORES)))

    out = np.zeros((2, S, D), dtype=np.float32)
    for b in range(2):
        acc = np.zeros((D, S), dtype=np.float32)
        for i in range(4 * b, 4 * b + 4):
            num = res.results[i][names["num"]]   # [2, 2, 128, 2048]
            den = res.results[i][names["den"]]   # [2, 4, 512]
            for j in range(2):
                acc += num[j].reshape(D, S) / den[j].reshape(S)[None, :]
        out[b] = acc.T + bo[None, :]
    return out
